# revision 1
# baseline (speedup 1.0000x reference)
"""Trainium2 Bass kernel for HeavilyCompressedAttention.

Sharding: 16 heads across 8 cores (2 heads/core, tensor-parallel);
compressed-KV path (single shared head) replicated on every core;
out_proj row-parallel with host-side partial sum.

Per-core device pipeline (all matmuls bf16, fp32 accumulation):
  P1: q/lk/lv/compress-score projections (T-stationary = hidden^T tiles
      streamed from HBM), fused RMSNorm + partial RoPE, block-softmax of
      compressor scores.
  P2: learned-weighted KV compression (entries) + shared ck/cv head
      (+RMSNorm/RoPE on ck).
  P3: per s-tile attention: banded local scores + compressed scores +
      sink, masked exp (no max pass -- logits bounded by RMSNorm),
      probability transposes on PE, ctx^T accumulation.
  P4: out-projection (row-parallel partial), PSUM->HBM direct store.
"""

import os
import sys

import numpy as np
import ml_dtypes

for _p in ("/opt/trn_rl_repo", "/root/.axon_site/_ro/trn_rl_repo"):
    if os.path.isdir(_p) and _p not in sys.path:
        sys.path.insert(0, _p)

from concourse import bacc, mybir  # noqa: E402
import concourse.tile as tile  # noqa: E402
from concourse.bass_utils import run_bass_kernel_spmd  # noqa: E402
from concourse.masks import make_identity  # noqa: E402

F32 = mybir.dt.float32
BF16 = mybir.dt.bfloat16
NPBF = ml_dtypes.bfloat16

S = 2048
HID = 2048
NH = 16
HD = 128
R = 16
C = S // R  # 128
WIN = 128
ROPE = HD // 2  # 64
HALF = ROPE // 2  # 32
EPS = 1e-6
NT = S // 128  # 16 s-tiles
KT = HID // 128  # 16 k-tiles
NCORES = 8
HPC = NH // NCORES  # 2 heads per core
SCALE = 1.0 / float(np.sqrt(HD))
MASKV = -30000.0

_CACHE = {}


def _build_bass():
    nc = bacc.Bacc("TRN2", target_bir_lowering=False, debug=False,
                   num_devices=NCORES)

    din = {}

    def inp(name, shape, dt):
        din[name] = nc.dram_tensor(name, list(shape), dt, kind="ExternalInput")
        return din[name]

    hT = inp("hT", [KT, NT, 128, 128], BF16)      # hidden^T tiles [k,i,hid,s]
    hN = inp("hN", [NT, 128, HID], BF16)          # hidden natural s-tiles
    wqlk = inp("wqlk", [128, KT, 512], BF16)      # [q0|q1|lk0|lk1] col-slices
    wlvc = inp("wlvc", [128, KT, 257], BF16)      # [lv0|lv1|Wc]
    wkv = inp("wkv", [128, KT, 256], BF16)        # [Wk|Wv] shared head
    wo = inp("wo", [128, HPC, HID], BF16)         # Wo rows per head [d,h,o]
    b_qlk = inp("b_qlk", [1, 512], BF16)
    b_lvc = inp("b_lvc", [1, 257], BF16)
    b_kv = inp("b_kv", [1, 256], BF16)
    tA = inp("tA", [128, NT, 64], F32)            # rope tables (q|k), w-folded
    tB = inp("tB", [128, NT, 64], F32)
    tC = inp("tC", [128, NT, 64], F32)
    tD = inp("tD", [128, NT, 64], F32)
    qk_pass = inp("qk_pass", [128, 128], F32)     # [qn_w|kn_w][64:] bcast rows
    ctA = inp("ctA", [C, HALF], F32)              # ck rope tables (block_ends)
    ctB = inp("ctB", [C, HALF], F32)
    ctC = inp("ctC", [C, HALF], F32)
    ctD = inp("ctD", [C, HALF], F32)
    ck_pass = inp("ck_pass", [C, ROPE], F32)      # kn_w[64:] bcast rows
    maskB = inp("maskB", [128, 256], F32)         # banded local mask (i>=1)
    mask0 = inp("mask0", [128, 256], F32)         # first-tile local mask
    m_all = inp("m_all", [128, 249], F32)         # sliding compressed mask
    dwide = inp("dwide", [128, 248], BF16)        # sliding block-diag pattern
    sinkk = inp("sinkk", [HPC, 128], BF16)        # sink_k per head
    sinkv = inp("sinkv", [64, HPC * 128], BF16)   # row0 = sink_v[h], rest 0

    out_p = nc.dram_tensor("out_p", [S, HID], F32, kind="ExternalOutput")

    with tile.TileContext(nc) as tc:
        with (
            tc.tile_pool(name="const", bufs=1) as cst,
            tc.tile_pool(name="persist", bufs=1) as per,
            tc.tile_pool(name="stream", bufs=6) as stm,
            tc.tile_pool(name="scratch", bufs=3) as scr,
            tc.tile_pool(name="stats", bufs=6) as sts,
        ):
            # ---- load constants ----
            def load(name, shape, dt):
                t = cst.tile(list(shape), dt, name=f"c_{name}")
                nc.sync.dma_start(out=t[:], in_=din[name].ap())
                return t

            wqlk_sb = load("wqlk", [128, KT, 512], BF16)
            wlvc_sb = load("wlvc", [128, KT, 257], BF16)
            wkv_sb = load("wkv", [128, KT, 256], BF16)
            wo_sb = load("wo", [128, HPC, HID], BF16)
            b_qlk_sb = load("b_qlk", [1, 512], BF16)
            b_lvc_sb = load("b_lvc", [1, 257], BF16)
            b_kv_sb = load("b_kv", [1, 256], BF16)
            tA_sb = load("tA", [128, NT, 64], F32)
            tB_sb = load("tB", [128, NT, 64], F32)
            tC_sb = load("tC", [128, NT, 64], F32)
            tD_sb = load("tD", [128, NT, 64], F32)
            qk_pass_sb = load("qk_pass", [128, 128], F32)
            ctA_sb = load("ctA", [C, HALF], F32)
            ctB_sb = load("ctB", [C, HALF], F32)
            ctC_sb = load("ctC", [C, HALF], F32)
            ctD_sb = load("ctD", [C, HALF], F32)
            ck_pass_sb = load("ck_pass", [C, ROPE], F32)
            maskB_sb = load("maskB", [128, 256], F32)
            mask0_sb = load("mask0", [128, 256], F32)
            m_all_sb = load("m_all", [128, 249], F32)
            dwide_sb = load("dwide", [128, 248], BF16)
            sinkv_sb = load("sinkv", [64, HPC * 128], BF16)

            ident_bf = cst.tile([128, 128], BF16)
            make_identity(nc, ident_bf[:])
            ident_f32 = cst.tile([128, 128], F32)
            make_identity(nc, ident_f32[:])
            ones1 = cst.tile([1, 128], BF16)
            nc.vector.memset(ones1[:], 1.0)
            eps_t = cst.tile([128, 1], F32)
            nc.vector.memset(eps_t[:], EPS)

            # ---- persistent activations ----
            qlkn = per.tile([128, NT, 512], BF16)   # roped q|lk (natural)
            lvn = per.tile([128, NT, 256], BF16)    # local v (natural)
            lkT = per.tile([128, NT + 1, 256], BF16)  # lk^T tiles, slot0=0
            cwN = per.tile([128, NT], F32)          # compressor scores
            wN = per.tile([128, NT], F32)           # block-softmaxed weights
            entries = per.tile([C, HID], BF16)
            eT = per.tile([128, KT, C], BF16)       # entries^T tiles
            cvn = per.tile([C, 128], BF16)
            ckT_aug = per.tile([128, HPC, 129], BF16)  # ck^T | sink_k col
            mgT = per.tile([128, HPC, S], BF16)     # merged^T = 0.5*(cc+cl)

            nc.vector.memset(lkT[:, 0, :], 0.0)
            for h in range(HPC):
                nc.sync.dma_start(out=ckT_aug[:, h, 128:129],
                                  in_=din["sinkk"].ap()[h].unsqueeze(1))

            # ================= P1: projections + norm/rope =================
            with (
                tc.tile_pool(name="ps_qlk", bufs=2, space="PSUM") as pq,
                tc.tile_pool(name="ps_lvc", bufs=2, space="PSUM") as pl,
                tc.tile_pool(name="ps_tp", bufs=2, space="PSUM") as ptp,
            ):
                for i in range(NT):
                    ps_q = pq.tile([128, 512], F32)
                    ps_l = pl.tile([128, 257], F32)
                    for k in range(KT):
                        hT_t = stm.tile([128, 128], BF16, tag="hT")
                        nc.sync.dma_start(out=hT_t[:], in_=hT.ap()[k, i])
                        nc.tensor.matmul(ps_q[:], hT_t[:], wqlk_sb[:, k, :],
                                         start=(k == 0), stop=False)
                        nc.tensor.matmul(ps_l[:], hT_t[:], wlvc_sb[:, k, :],
                                         start=(k == 0), stop=False)
                    nc.tensor.matmul(ps_q[:], ones1[:], b_qlk_sb[:],
                                     start=False, stop=True)
                    nc.tensor.matmul(ps_l[:], ones1[:], b_lvc_sb[:],
                                     start=False, stop=True)

                    # rms norm (over d) for the 4 sub-tensors [q0|q1|k0|k1]
                    ssq = sts.tile([128, 4], F32)
                    for j in range(4):
                        sq_s = scr.tile([128, 128], F32, tag="sq")
                        nc.scalar.activation(
                            sq_s[:], ps_q[:, j * 128:(j + 1) * 128],
                            mybir.ActivationFunctionType.Square,
                            accum_out=ssq[:, j:j + 1])
                    rms = sts.tile([128, 4], F32)
                    nc.scalar.activation(rms[:], ssq[:],
                                         mybir.ActivationFunctionType.Sqrt,
                                         scale=1.0 / HD, bias=eps_t[:])
                    rinv = sts.tile([128, 4], F32)
                    nc.vector.reciprocal(rinv[:], rms[:])

                    qn = qlkn[:, i, :]
                    nc.vector.tensor_mul(
                        qn.rearrange("p (a b) -> p a b", a=4),
                        ps_q[:].rearrange("p (a b) -> p a b", a=4),
                        rinv[:].unsqueeze(2).broadcast_to([128, 4, 128]))
                    # partial rope on cols [0:64) of each sub-tensor;
                    # 4-D views: [p, sect(q/k), head, cols]
                    qn4 = qn.rearrange("p (s r b) -> p s r b", s=2, r=2)
                    x1 = qn4[:, :, :, 0:HALF]
                    x2 = qn4[:, :, :, HALF:ROPE]
                    xp = qn4[:, :, :, ROPE:128]

                    def tslice(t):
                        return (t[:, i, :]
                                .rearrange("p (s c) -> p s c", s=2)
                                .unsqueeze(2)
                                .broadcast_to([128, 2, 2, HALF]))

                    t1 = scr.tile([128, 4, HALF], BF16, tag="t1")
                    t2 = scr.tile([128, 4, HALF], BF16, tag="t2")
                    t3 = scr.tile([128, 4, HALF], BF16, tag="t3")
                    t4 = scr.tile([128, 4, HALF], BF16, tag="t4")

                    def v4(t):
                        return t[:].rearrange("p (s r) c -> p s r c", s=2)

                    nc.vector.tensor_mul(v4(t1), x1, tslice(tA_sb))
                    nc.vector.tensor_mul(v4(t2), x2, tslice(tB_sb))
                    nc.vector.tensor_mul(v4(t3), x1, tslice(tC_sb))
                    nc.vector.tensor_mul(v4(t4), x2, tslice(tD_sb))
                    nc.vector.tensor_sub(x1, v4(t1), v4(t2))
                    nc.vector.tensor_add(x2, v4(t3), v4(t4))
                    # passthrough cols [64:128) *= norm weight tail
                    nc.vector.tensor_mul(
                        xp,
                        xp,
                        qk_pass_sb[:].rearrange("p (s c) -> p s c", s=2)
                        .unsqueeze(2).broadcast_to([128, 2, 2, ROPE]))

                    # collect compressor scores + local V
                    nc.scalar.copy(cwN[:, i:i + 1], ps_l[:, 256:257])
                    nc.scalar.copy(lvn[:, i, :], ps_l[:, 0:256])

                # block-softmax of compressor scores (R=16 blocks)
                ps_cw1 = ptp.tile([16, 128], F32, tag="tp_cw1")
                nc.tensor.transpose(ps_cw1[:], cwN[:], ident_f32[:])
                cwT = scr.tile([16, 128], F32, tag="cwT")
                nc.scalar.copy(cwT[:], ps_cw1[:])
                cw3 = cwT[:].rearrange("p (g r) -> p g r", g=8)
                cmx = sts.tile([16, 8], F32)
                nc.vector.tensor_reduce(cmx[:], cw3, mybir.AxisListType.X,
                                        mybir.AluOpType.max)
                cwE = scr.tile([16, 128], F32, tag="cwE")
                nc.vector.tensor_sub(
                    cwE[:].rearrange("p (g r) -> p g r", g=8), cw3,
                    cmx[:].unsqueeze(2).broadcast_to([16, 8, 16]))
                nc.scalar.activation(cwE[:], cwE[:],
                                     mybir.ActivationFunctionType.Exp)
                csum = sts.tile([16, 8], F32)
                nc.vector.tensor_reduce(
                    csum[:], cwE[:].rearrange("p (g r) -> p g r", g=8),
                    mybir.AxisListType.X, mybir.AluOpType.add)
                crec = sts.tile([16, 8], F32)
                nc.vector.reciprocal(crec[:], csum[:])
                cwW = scr.tile([16, 128], F32, tag="cwW")
                nc.vector.tensor_mul(
                    cwW[:].rearrange("p (g r) -> p g r", g=8),
                    cwE[:].rearrange("p (g r) -> p g r", g=8),
                    crec[:].unsqueeze(2).broadcast_to([16, 8, 16]))
                ps_cw2 = ptp.tile([128, 16], F32, tag="tp_cw2")
                nc.tensor.transpose(ps_cw2[:], cwW[:], ident_f32[0:16, 0:16])
                nc.scalar.copy(wN[:], ps_cw2[:])

            # ================= P2: entries + ck/cv =================
            with (
                tc.tile_pool(name="ps_e", bufs=1, space="PSUM") as pe,
                tc.tile_pool(name="ps_kv", bufs=1, space="PSUM") as pkv,
                tc.tile_pool(name="ps_tp2", bufs=2, space="PSUM") as ptp2,
            ):
                ps_e = pe.tile([C, HID], F32)
                for i in range(NT):
                    wbig = scr.tile([128, 128], BF16, tag="wbig")
                    nc.vector.tensor_scalar_mul(
                        wbig[:], dwide_sb[:, 120 - 8 * i:248 - 8 * i],
                        wN[:, i:i + 1])
                    hN_t = stm.tile([128, HID], BF16, tag="hN", bufs=3)
                    nc.sync.dma_start(out=hN_t[:], in_=hN.ap()[i])
                    for hc in range(4):
                        nc.tensor.matmul(ps_e[:, hc * 512:(hc + 1) * 512],
                                         wbig[:], hN_t[:, hc * 512:(hc + 1) * 512],
                                         start=(i == 0), stop=(i == NT - 1))
                for hc in range(4):
                    nc.scalar.copy(entries[:, hc * 512:(hc + 1) * 512],
                                   ps_e[:, hc * 512:(hc + 1) * 512])
                for k in range(KT):
                    ps_t = ptp2.tile([128, 128], BF16, tag="tp_e")
                    nc.tensor.transpose(ps_t[:],
                                        entries[:, k * 128:(k + 1) * 128],
                                        ident_bf[:])
                    nc.scalar.copy(eT[:, k, :], ps_t[:])

                ps_kv = pkv.tile([C, 256], F32)
                for k in range(KT):
                    nc.tensor.matmul(ps_kv[:], eT[:, k, :], wkv_sb[:, k, :],
                                     start=(k == 0), stop=False)
                nc.tensor.matmul(ps_kv[:], ones1[:], b_kv_sb[:],
                                 start=False, stop=True)

                # ck: rmsnorm + rope at block-end positions
                ssqc = sts.tile([C, 1], F32)
                sq_c = scr.tile([C, 128], F32, tag="sq")
                nc.scalar.activation(sq_c[:], ps_kv[:, 0:128],
                                     mybir.ActivationFunctionType.Square,
                                     accum_out=ssqc[:])
                rmsc = sts.tile([C, 1], F32)
                nc.scalar.activation(rmsc[:], ssqc[:],
                                     mybir.ActivationFunctionType.Sqrt,
                                     scale=1.0 / HD, bias=eps_t[:])
                rinvc = sts.tile([C, 1], F32)
                nc.vector.reciprocal(rinvc[:], rmsc[:])
                ckn = scr.tile([C, 128], F32, tag="ckn")
                nc.vector.tensor_scalar_mul(ckn[:], ps_kv[:, 0:128], rinvc[:])
                ckR = scr.tile([C, 128], BF16, tag="ckR")
                ct1 = scr.tile([C, HALF], F32, tag="ct1")
                ct2 = scr.tile([C, HALF], F32, tag="ct2")
                nc.vector.tensor_mul(ct1[:], ckn[:, 0:HALF], ctA_sb[:])
                nc.vector.tensor_mul(ct2[:], ckn[:, HALF:ROPE], ctB_sb[:])
                nc.vector.tensor_sub(ckR[:, 0:HALF], ct1[:], ct2[:])
                nc.vector.tensor_mul(ct1[:], ckn[:, 0:HALF], ctC_sb[:])
                nc.vector.tensor_mul(ct2[:], ckn[:, HALF:ROPE], ctD_sb[:])
                nc.vector.tensor_add(ckR[:, HALF:ROPE], ct1[:], ct2[:])
                nc.vector.tensor_mul(ckR[:, ROPE:128], ckn[:, ROPE:128],
                                     ck_pass_sb[:])
                nc.scalar.copy(cvn[:], ps_kv[:, 128:256])
                ps_ct = ptp2.tile([128, 128], BF16, tag="tp_e")
                nc.tensor.transpose(ps_ct[:], ckR[:], ident_bf[:])
                for h in range(HPC):
                    nc.scalar.copy(ckT_aug[:, h, 0:128], ps_ct[:])

            # ================= P3: attention =================
            with (
                tc.tile_pool(name="ps_sc", bufs=2, space="PSUM") as psc,
                tc.tile_pool(name="ps_ctx", bufs=2, space="PSUM") as pcx,
                tc.tile_pool(name="ps_tp3", bufs=4, space="PSUM") as ptp3,
            ):
                for i in range(NT):
                    qTs = []
                    for h in range(HPC):
                        ps_tq = ptp3.tile([128, 128], BF16, tag="tp3")
                        nc.tensor.transpose(
                            ps_tq[:], qlkn[:, i, h * 128:(h + 1) * 128],
                            ident_bf[:])
                        qT = scr.tile([128, 128], BF16, tag="qT", bufs=4)
                        nc.scalar.copy(qT[:], ps_tq[:])
                        qTs.append(qT)
                        ps_tk = ptp3.tile([128, 128], BF16, tag="tp3")
                        nc.tensor.transpose(
                            ps_tk[:],
                            qlkn[:, i, 256 + h * 128:256 + (h + 1) * 128],
                            ident_bf[:])
                        nc.scalar.copy(lkT[:, i + 1, h * 128:(h + 1) * 128],
                                       ps_tk[:])
                    for h in range(HPC):
                        ps_s = psc.tile([128, 448], F32)
                        nc.tensor.matmul(
                            ps_s[:, 0:256], qTs[h][:],
                            lkT[:, i:i + 2, h * 128:(h + 1) * 128],
                            start=True, stop=True)
                        nc.tensor.matmul(ps_s[:, 256:385], qTs[h][:],
                                         ckT_aug[:, h, :],
                                         start=True, stop=True)
                        nc.vector.memset(ps_s[:, 385:448], MASKV)
                        lm = mask0_sb if i == 0 else maskB_sb
                        nc.vector.tensor_add(ps_s[:, 0:256], ps_s[:, 0:256],
                                             lm[:])
                        nc.vector.tensor_add(
                            ps_s[:, 256:384], ps_s[:, 256:384],
                            m_all_sb[:, 120 - 8 * i:248 - 8 * i])
                        # two separate softmaxes: local (cols 0:256) and
                        # compressed+sink (cols 256:448)
                        p_t = scr.tile([128, 448], BF16, tag="p", bufs=3)
                        den = sts.tile([128, 2], F32)
                        nc.scalar.activation(p_t[:, 0:256], ps_s[:, 0:256],
                                             mybir.ActivationFunctionType.Exp,
                                             scale=SCALE,
                                             accum_out=den[:, 0:1])
                        nc.scalar.activation(p_t[:, 256:448], ps_s[:, 256:448],
                                             mybir.ActivationFunctionType.Exp,
                                             scale=SCALE,
                                             accum_out=den[:, 1:2])
                        rden = sts.tile([128, 2], F32)
                        nc.vector.reciprocal(rden[:], den[:])
                        pn = scr.tile([128, 448], BF16, tag="pn", bufs=3)
                        nc.vector.tensor_scalar_mul(pn[:, 0:256], p_t[:, 0:256],
                                                    rden[:, 0:1])
                        nc.vector.tensor_scalar_mul(pn[:, 256:448],
                                                    p_t[:, 256:448],
                                                    rden[:, 1:2])

                        ps_c = pcx.tile([128, 128], F32)
                        chunks = range(1, 4) if i == 0 else range(4)
                        first = True
                        for cidx in chunks:
                            w64 = 64 if cidx == 3 else 128
                            ps_tp_t = ptp3.tile([128, 128], BF16, tag="tp3")
                            nc.tensor.transpose(
                                ps_tp_t[0:w64, :],
                                pn[:, cidx * 128:cidx * 128 + w64],
                                ident_bf[:])
                            pT_sb = scr.tile([128, 128], BF16, tag="pT",
                                             bufs=6)
                            nc.scalar.copy(pT_sb[0:w64, :], ps_tp_t[0:w64, :])
                            if cidx == 0:
                                st = lvn[:, i - 1, h * 128:(h + 1) * 128]
                            elif cidx == 1:
                                st = lvn[:, i, h * 128:(h + 1) * 128]
                            elif cidx == 2:
                                st = cvn[:]
                            else:
                                st = sinkv_sb[:, h * 128:(h + 1) * 128]
                            nc.tensor.matmul(ps_c[:], st, pT_sb[0:w64, :],
                                             start=first, stop=(cidx == 3))
                            first = False
                        nc.scalar.activation(mgT[:, h, i * 128:(i + 1) * 128],
                                             ps_c[:],
                                             mybir.ActivationFunctionType.Copy,
                                             scale=0.5)

            # ================= P4: out projection =================
            with tc.tile_pool(name="ps_out", bufs=2, space="PSUM") as pout:
                for i in range(NT):
                    ps_o = pout.tile([128, HID], F32)
                    for h in range(HPC):
                        for oc in range(4):
                            nc.tensor.matmul(
                                ps_o[:, oc * 512:(oc + 1) * 512],
                                mgT[:, h, i * 128:(i + 1) * 128],
                                wo_sb[:, h, oc * 512:(oc + 1) * 512],
                                start=(h == 0), stop=(h == HPC - 1))
                    o_sb = scr.tile([128, HID], F32, tag="o_sb", bufs=2)
                    nc.scalar.copy(o_sb[:, 0:HID // 2], ps_o[:, 0:HID // 2])
                    nc.vector.tensor_copy(o_sb[:, HID // 2:], ps_o[:, HID // 2:])
                    nc.sync.dma_start(out=out_p.ap()[i * 128:(i + 1) * 128, :],
                                      in_=o_sb[:])

    nc.compile()
    return nc


def _host_prep(inputs):
    """Build the 8 per-core input maps from full inputs."""
    hs = np.asarray(inputs["hidden_states"], np.float32)[0]  # [S, HID]
    Wq = np.asarray(inputs["Wq"], np.float32)
    Wc = np.asarray(inputs["Wc"], np.float32)
    Wk = np.asarray(inputs["Wk"], np.float32)
    Wv = np.asarray(inputs["Wv"], np.float32)
    Wlk = np.asarray(inputs["Wlk"], np.float32)
    Wlv = np.asarray(inputs["Wlv"], np.float32)
    qn_w = np.asarray(inputs["qn_w"], np.float32)
    kn_w = np.asarray(inputs["kn_w"], np.float32)
    sink_k = np.asarray(inputs["sink_k"], np.float32)
    sink_v = np.asarray(inputs["sink_v"], np.float32)
    Wo = np.asarray(inputs["Wo"], np.float32)
    bq = np.asarray(inputs["bq"], np.float32)
    bc = np.asarray(inputs["bc"], np.float32)
    bk = np.asarray(inputs["bk"], np.float32)
    bv = np.asarray(inputs["bv"], np.float32)
    blk = np.asarray(inputs["blk"], np.float32)
    blv = np.asarray(inputs["blv"], np.float32)

    hT_t = np.ascontiguousarray(
        hs.T.reshape(KT, 128, NT, 128).transpose(0, 2, 1, 3)).astype(NPBF)
    hN_t = hs.reshape(NT, 128, HID).astype(NPBF)

    def dev_w(w):  # [HID, F] -> [128, KT, F]
        return np.ascontiguousarray(
            w.reshape(KT, 128, -1).transpose(1, 0, 2)).astype(NPBF)

    # rope tables for token positions (q & k variants, norm weight folded)
    pos = np.arange(S, dtype=np.float32)
    inv_freq = 1.0 / (10000.0 ** (np.arange(HALF, dtype=np.float32) * 2.0 / ROPE))
    ang = pos[:, None] * inv_freq[None, :]
    cos, sin = np.cos(ang), np.sin(ang)  # [S, HALF]

    def fold(tab, w_half):
        return tab * w_half[None, :]

    def pack(tq, tk):  # [S,HALF]x2 -> [128, NT, 64]
        t = np.concatenate([tq, tk], axis=1)  # [S, 64]
        return np.ascontiguousarray(
            t.reshape(NT, 128, 64).transpose(1, 0, 2)).astype(np.float32)

    qw1, qw2 = qn_w[0:HALF], qn_w[HALF:ROPE]
    kw1, kw2 = kn_w[0:HALF], kn_w[HALF:ROPE]
    tA = pack(fold(cos, qw1), fold(cos, kw1))
    tB = pack(fold(sin, qw2), fold(sin, kw2))
    tC = pack(fold(sin, qw1), fold(sin, kw1))
    tD = pack(fold(cos, qw2), fold(cos, kw2))
    qk_pass = np.broadcast_to(
        np.concatenate([qn_w[ROPE:], kn_w[ROPE:]])[None, :],
        (128, 128)).astype(np.float32).copy()

    # ck rope tables at block-end positions
    pos_c = (np.arange(C, dtype=np.float32) * R + (R - 1))
    angc = pos_c[:, None] * inv_freq[None, :]
    cosc, sinc = np.cos(angc), np.sin(angc)
    ctA = (cosc * kw1[None, :]).astype(np.float32)
    ctB = (sinc * kw2[None, :]).astype(np.float32)
    ctC = (sinc * kw1[None, :]).astype(np.float32)
    ctD = (cosc * kw2[None, :]).astype(np.float32)
    ck_pass = np.broadcast_to(kn_w[ROPE:][None, :],
                              (C, ROPE)).astype(np.float32).copy()

    # masks
    r = np.arange(128)[:, None]
    j = np.arange(256)[None, :]
    maskB = np.where((j >= r) & (j <= r + 128), 0.0, MASKV).astype(np.float32)
    mask0 = np.where((j >= 128) & (j - 128 <= r), 0.0, MASKV).astype(np.float32)
    idx = np.arange(249)[None, :]
    m_all = np.where(16 * (idx - 120) + 15 <= r, 0.0, MASKV).astype(np.float32)
    idx2 = np.arange(248)[None, :]
    dwide = (idx2 == 120 + r // 16).astype(np.float32).astype(NPBF)

    common = dict(hT=hT_t, hN=hN_t, tA=tA, tB=tB, tC=tC, tD=tD,
                  qk_pass=qk_pass, ctA=ctA, ctB=ctB, ctC=ctC, ctD=ctD,
                  ck_pass=ck_pass, maskB=maskB, mask0=mask0, m_all=m_all,
                  dwide=dwide,
                  wkv=dev_w(np.concatenate([Wk, Wv], axis=1)),
                  b_kv=np.concatenate([bk, bv])[None, :].astype(NPBF))

    Wq4 = Wq.reshape(HID, NH, HD)
    Wlk4 = Wlk.reshape(HID, NH, HD)
    Wlv4 = Wlv.reshape(HID, NH, HD)
    bq4 = bq.reshape(NH, HD)
    blk4 = blk.reshape(NH, HD)
    blv4 = blv.reshape(NH, HD)
    Wo4 = Wo.reshape(NH, HD, HID)

    in_maps = []
    for c in range(NCORES):
        hh = [HPC * c + h for h in range(HPC)]
        wqlk = np.concatenate([Wq4[:, hh[0]], Wq4[:, hh[1]],
                               Wlk4[:, hh[0]], Wlk4[:, hh[1]]], axis=1)
        wlvc = np.concatenate([Wlv4[:, hh[0]], Wlv4[:, hh[1]], Wc], axis=1)
        b_qlk = np.concatenate([bq4[hh[0]], bq4[hh[1]],
                                blk4[hh[0]], blk4[hh[1]]])[None, :]
        b_lvc = np.concatenate([blv4[hh[0]], blv4[hh[1]], bc])[None, :]
        wo_c = np.ascontiguousarray(
            Wo4[hh].transpose(1, 0, 2)).astype(NPBF)  # [128, HPC, HID]
        sinkk = sink_k[hh].astype(NPBF)
        sinkv = np.zeros((64, HPC * 128), np.float32)
        for h in range(HPC):
            sinkv[0, h * 128:(h + 1) * 128] = sink_v[hh[h]]
        m = dict(common)
        m.update(wqlk=dev_w(wqlk), wlvc=dev_w(wlvc),
                 b_qlk=b_qlk.astype(NPBF), b_lvc=b_lvc.astype(NPBF),
                 wo=wo_c, sinkk=sinkk, sinkv=sinkv.astype(NPBF))
        in_maps.append(m)
    return in_maps


def kernel(**inputs):
    if "nc" not in _CACHE:
        _CACHE["nc"] = _build_bass()
    nc = _CACHE["nc"]
    in_maps = _host_prep(inputs)
    res = run_bass_kernel_spmd(nc, in_maps, core_ids=list(range(NCORES)))
    out = np.zeros((S, HID), np.float64)
    for c in range(NCORES):
        out += res.results[c]["out_p"].astype(np.float64)
    out += np.asarray(inputs["bo"], np.float32)[None, :]
    return out[None].astype(np.float32)



# revision 23
# speedup vs baseline: 2.0217x; 2.0217x over previous
"""Trainium2 Bass kernel for HeavilyCompressedAttention.

Sharding: 16 heads across 8 cores (2 heads/core, tensor-parallel);
compressed-KV path (single shared head) replicated on every core;
out_proj row-parallel with host-side partial sum (bf16 partials).

v2: weight DMAs split per k-group for early P1 start, hN prefetched,
qkT/pT transposes batched through one PSUM tile with single wide
copies, combined per-i mask table, sink handled as rank-1 matmul,
SBUF-only elementwise ops on GpSimd, P4 fused into the P3 loop,
zero-bias fast path.
"""

import os
import sys

import numpy as np
import ml_dtypes

for _p in ("/opt/trn_rl_repo", "/root/.axon_site/_ro/trn_rl_repo"):
    if os.path.isdir(_p) and _p not in sys.path:
        sys.path.insert(0, _p)

from concourse import bacc, mybir  # noqa: E402
import concourse.tile as tile  # noqa: E402
from concourse.bass_utils import run_bass_kernel_spmd  # noqa: E402
from concourse.masks import make_identity  # noqa: E402

F32 = mybir.dt.float32
BF16 = mybir.dt.bfloat16
NPBF = ml_dtypes.bfloat16

S = 2048
HID = 2048
NH = 16
HD = 128
R = 16
C = S // R  # 128
WIN = 128
ROPE = HD // 2  # 64
HALF = ROPE // 2  # 32
EPS = 1e-6
NT = S // 128  # 16 s-tiles
KT = HID // 128  # 16 k-tiles
NCORES = 8
HPC = NH // NCORES  # 2 heads per core
SCALE = 1.0 / float(np.sqrt(HD))
MASKV = -30000.0
KG = 4  # k-tiles per weight-chunk DMA

_CACHE = {}

# ---- const blob column layout (bf16 [128, CB]) ----
_OFF = {}
_cb = 0


def _col(name, n):
    global _cb
    _OFF[name] = (_cb, _cb + n)
    _cb += n


_col("wg0", KG * 769)       # per k: [wqlk_k (512) | wlvc_k (257)]
_col("tA", NT * 64)         # --- SMALL chunk (needed early in P1) ---
_col("tB", NT * 64)
_col("tC", NT * 64)
_col("tD", NT * 64)
_col("qk_pass", 128)
_col("ctA", HALF)           # rows 0:C
_col("ctB", HALF)
_col("ctC", HALF)
_col("ctD", HALF)
_col("ck_pass", ROPE)       # rows 0:C
_col("dwide", 248)
_col("b_qlk", 512)          # row 0
_col("b_lvc", 257)          # row 0
_col("b_kv", 256)           # row 0
_col("sinkkT", HPC)         # [128, HPC] column per head
_col("sinkv", HPC * 128)    # row 0 only used (rank-1 matmul)
for _g in range(1, KT // KG):
    _col(f"wg{_g}", KG * 769)
_col("wkv", KT * 256)       # --- BIG tail (needed from P2 on) ---
_col("wo", HPC * HID)       # [p, h, HID]  (0.5 folded in)
_col("mcomb", NT * 385)     # per-i [local 256 | comp 128 | sink 0] mask
CB = _cb
_SMALL = (_OFF["tA"][0], _OFF["sinkv"][1])
_BIG = (_OFF["wkv"][0], CB)


def _build_bass(zero_bias):
    nc = bacc.Bacc("TRN2", target_bir_lowering=False, debug=False,
                   num_devices=NCORES)

    din = {}

    def inp(name, shape, dt):
        din[name] = nc.dram_tensor(name, list(shape), dt, kind="ExternalInput")
        return din[name]

    hT = inp("hT", [NT, 128, KT * 128], BF16)   # [i][hid_p][k*128+c(s)]
    hN = inp("hN", [NT // 2, 128, 2 * HID], BF16)  # [j][s_p][jj*HID+hid]
    cblob = inp("cblob", [128, CB], BF16)

    out_p = nc.dram_tensor("out_p", [NT, 128, HID], BF16,
                           kind="ExternalOutput")

    with tile.TileContext(nc) as tc:
        with (
            tc.tile_pool(name="const", bufs=1) as cst,
            tc.tile_pool(name="persist", bufs=1) as per,
            tc.tile_pool(name="stream", bufs=3) as stm,
            tc.tile_pool(name="scratch", bufs=3) as scr,
            tc.tile_pool(name="stats", bufs=6) as sts,
        ):
            blob = cst.tile([128, CB], BF16, name="c_blob")
            # wg0 + small tables first (P1 start), wg1..3 next;
            # big tail (wkv|wo|mcomb) emitted just before P2
            a, b = _OFF["wg0"]
            nc.sync.dma_start(out=blob[:, a:b], in_=cblob.ap()[:, a:b])
            nc.sync.dma_start(out=blob[:, _SMALL[0]:_SMALL[1]],
                              in_=cblob.ap()[:, _SMALL[0]:_SMALL[1]])
            for g in range(1, KT // KG):
                a, b = _OFF[f"wg{g}"]
                nc.sync.dma_start(out=blob[:, a:b],
                                  in_=cblob.ap()[:, a:b])

            def cv(name):
                a, b = _OFF[name]
                return blob[:, a:b]

            # weight views: wqlk k -> wg{k//KG} cols [(k%KG)*769, +512]
            def wqlk_k(k):
                base = _OFF[f"wg{k // KG}"][0] + (k % KG) * 769
                return blob[:, base:base + 512]

            def wlvc_k(k):
                base = _OFF[f"wg{k // KG}"][0] + (k % KG) * 769 + 512
                return blob[:, base:base + 257]

            wkv_sb = cv("wkv").rearrange("p (k n) -> p k n", k=KT)
            wo_sb = cv("wo").rearrange("p (h n) -> p h n", h=HPC)
            tA_sb = cv("tA").rearrange("p (i n) -> p i n", i=NT)
            tB_sb = cv("tB").rearrange("p (i n) -> p i n", i=NT)
            tC_sb = cv("tC").rearrange("p (i n) -> p i n", i=NT)
            tD_sb = cv("tD").rearrange("p (i n) -> p i n", i=NT)
            qk_pass_sb = cv("qk_pass")
            ctA_sb = cv("ctA")[0:C, :]
            ctB_sb = cv("ctB")[0:C, :]
            ctC_sb = cv("ctC")[0:C, :]
            ctD_sb = cv("ctD")[0:C, :]
            ck_pass_sb = cv("ck_pass")[0:C, :]
            mcomb_sb = cv("mcomb").rearrange("p (i n) -> p i n", i=NT)
            dwide_sb = cv("dwide")
            b_qlk_sb = cv("b_qlk")[0:1, :]
            b_lvc_sb = cv("b_lvc")[0:1, :]
            b_kv_sb = cv("b_kv")[0:1, :]
            sinkkT_sb = cv("sinkkT")
            sinkv_sb = cv("sinkv")[0:1, :]

            ident_bf = cst.tile([128, 128], BF16)
            make_identity(nc, ident_bf[:])
            ident_f32 = cst.tile([128, 128], F32)
            make_identity(nc, ident_f32[:])
            ones1 = cst.tile([1, 128], BF16)
            nc.vector.memset(ones1[:], 1.0)
            eps_t = cst.tile([128, 1], F32)
            nc.vector.memset(eps_t[:], EPS)

            # ---- persistent activations ----
            lvn = per.tile([128, NT, 256], BF16)    # local v (natural)
            # q0|q1|lk0|lk1 transposed, slot i+1 = tile i; slot0 lk = 0
            qkT = per.tile([128, NT + 1, 512], BF16)
            cwN = per.tile([128, NT], F32)          # compressor scores
            wN = per.tile([128, NT], F32)           # block-softmaxed weights
            entries = per.tile([C, HID], BF16)
            eT = per.tile([128, KT, C], BF16)       # entries^T tiles
            cvn = per.tile([C, 128], BF16)
            ckT_aug = per.tile([128, HPC, 129], BF16)  # ck^T | sink_k col
            mgT = per.tile([128, HPC, S], BF16)     # merged^T = cc+cl

            nc.vector.memset(qkT[:, 0, 256:512], 0.0)
            for h in range(HPC):
                nc.vector.tensor_copy(ckT_aug[:, h, 128:129],
                                      sinkkT_sb[:, h:h + 1])

            # ================= P1: projections + norm/rope =================
            with (
                tc.tile_pool(name="ps_qlk", bufs=2, space="PSUM") as pq,
                tc.tile_pool(name="ps_lvc", bufs=2, space="PSUM") as pl,
                tc.tile_pool(name="ps_tp", bufs=1, space="PSUM") as ptp,
            ):
                for i in range(NT):
                    hT_t = stm.tile([128, KT, 128], BF16, tag="hT")
                    nc.sync.dma_start(out=hT_t[:], in_=hT.ap()[i])
                    ps_q = pq.tile([128, 512], F32)
                    ps_l = pl.tile([128, 257], F32)
                    for k in range(KT):
                        nc.tensor.matmul(ps_q[:], hT_t[:, k, :], wqlk_k(k),
                                         start=(k == 0),
                                         stop=(zero_bias and k == KT - 1))
                        nc.tensor.matmul(ps_l[:], hT_t[:, k, :], wlvc_k(k),
                                         start=(k == 0),
                                         stop=(zero_bias and k == KT - 1))
                    if not zero_bias:
                        nc.tensor.matmul(ps_q[:], ones1[:], b_qlk_sb[:],
                                         start=False, stop=True)
                        nc.tensor.matmul(ps_l[:], ones1[:], b_lvc_sb[:],
                                         start=False, stop=True)

                    # rms norm (over d) for the 4 sub-tensors [q0|q1|k0|k1]
                    ssq = sts.tile([128, 4], F32)
                    for j in range(4):
                        sq_s = scr.tile([128, 128], F32, tag="sq")
                        nc.scalar.activation(
                            sq_s[:], ps_q[:, j * 128:(j + 1) * 128],
                            mybir.ActivationFunctionType.Square,
                            accum_out=ssq[:, j:j + 1])
                    rms = sts.tile([128, 4], F32)
                    nc.scalar.activation(rms[:], ssq[:],
                                         mybir.ActivationFunctionType.Sqrt,
                                         scale=1.0 / HD, bias=eps_t[:])
                    rinv = sts.tile([128, 4], F32)
                    nc.vector.reciprocal(rinv[:], rms[:])

                    qlkn = scr.tile([128, 512], BF16, tag="qlkn", bufs=3)
                    qn = qlkn[:]
                    nc.vector.tensor_mul(
                        qn.rearrange("p (a b) -> p a b", a=4),
                        ps_q[:].rearrange("p (a b) -> p a b", a=4),
                        rinv[:].unsqueeze(2).broadcast_to([128, 4, 128]))
                    # partial rope on cols [0:64) of each sub-tensor;
                    # 4-D views: [p, sect(q/k), head, cols]
                    qn4 = qn.rearrange("p (s r b) -> p s r b", s=2, r=2)
                    x1 = qn4[:, :, :, 0:HALF]
                    x2 = qn4[:, :, :, HALF:ROPE]
                    xp = qn4[:, :, :, ROPE:128]

                    def tslice(t):
                        return (t[:, i, :]
                                .rearrange("p (s c) -> p s c", s=2)
                                .unsqueeze(2)
                                .broadcast_to([128, 2, 2, HALF]))

                    t1 = scr.tile([128, 4, HALF], BF16, tag="t1")
                    t2 = scr.tile([128, 4, HALF], BF16, tag="t2")
                    t3 = scr.tile([128, 4, HALF], BF16, tag="t3")
                    t4 = scr.tile([128, 4, HALF], BF16, tag="t4")

                    def v4(t):
                        return t[:].rearrange("p (s r) c -> p s r c", s=2)

                    nc.vector.tensor_mul(v4(t1), x1, tslice(tA_sb))
                    nc.vector.tensor_mul(v4(t2), x2, tslice(tB_sb))
                    nc.vector.tensor_mul(v4(t3), x1, tslice(tC_sb))
                    nc.vector.tensor_mul(v4(t4), x2, tslice(tD_sb))
                    nc.vector.tensor_sub(x1, v4(t1), v4(t2))
                    nc.vector.tensor_add(x2, v4(t3), v4(t4))
                    # passthrough cols [64:128) *= norm weight tail
                    nc.vector.tensor_mul(
                        xp,
                        xp,
                        qk_pass_sb.rearrange("p (s c) -> p s c", s=2)
                        .unsqueeze(2).broadcast_to([128, 2, 2, ROPE]))

                    # collect compressor scores + local V
                    nc.scalar.copy(cwN[:, i:i + 1], ps_l[:, 256:257])
                    nc.vector.tensor_copy(lvn[:, i, :], ps_l[:, 0:256])

                    # transpose q0|q1|lk0|lk1 -> qkT slot i+1 (one batch)
                    ps_t4 = ptp.tile([128, 512], BF16, tag="tp4")
                    for j in range(4):
                        nc.tensor.transpose(
                            ps_t4[:, j * 128:(j + 1) * 128],
                            qn[:, j * 128:(j + 1) * 128], ident_bf[:])
                    nc.scalar.copy(qkT[:, i + 1, :], ps_t4[:])

                # block-softmax of compressor scores (R=16 blocks)
                ps_cw1 = ptp.tile([16, 128], F32, tag="tp_cw1")
                nc.tensor.transpose(ps_cw1[:], cwN[:], ident_f32[:])
                cwT = scr.tile([16, 128], F32, tag="cwT")
                nc.scalar.copy(cwT[:], ps_cw1[:])
                cwE = scr.tile([16, 128], F32, tag="cwE")
                nc.scalar.activation(cwE[:], cwT[:],
                                     mybir.ActivationFunctionType.Exp)
                csum = sts.tile([16, 8], F32)
                nc.vector.tensor_reduce(
                    csum[:], cwE[:].rearrange("p (g r) -> p g r", g=8),
                    mybir.AxisListType.X, mybir.AluOpType.add)
                crec = sts.tile([16, 8], F32)
                nc.vector.reciprocal(crec[:], csum[:])
                cwW = scr.tile([16, 128], F32, tag="cwW")
                nc.vector.tensor_mul(
                    cwW[:].rearrange("p (g r) -> p g r", g=8),
                    cwE[:].rearrange("p (g r) -> p g r", g=8),
                    crec[:].unsqueeze(2).broadcast_to([16, 8, 16]))
                ps_cw2 = ptp.tile([128, 16], F32, tag="tp_cw2")
                nc.tensor.transpose(ps_cw2[:], cwW[:], ident_f32[0:16, 0:16])
                nc.scalar.copy(wN[:], ps_cw2[:])

            # big const tail + hN pairs (used by P2 on)
            nc.sync.dma_start(out=blob[:, _BIG[0]:_BIG[1]],
                              in_=cblob.ap()[:, _BIG[0]:_BIG[1]])
            hN_tiles = []
            for j in range(NT // 2):
                hN_t = stm.tile([128, 2, HID], BF16, tag="hN", bufs=4)
                nc.sync.dma_start(out=hN_t[:], in_=hN.ap()[j])
                hN_tiles.append(hN_t)

            # ================= P2: entries + ck/cv =================
            with (
                tc.tile_pool(name="ps_e", bufs=1, space="PSUM") as pe,
                tc.tile_pool(name="ps_kv", bufs=1, space="PSUM") as pkv,
                tc.tile_pool(name="ps_tp2", bufs=2, space="PSUM") as ptp2,
            ):
                ps_e = pe.tile([C, HID], F32)
                for j in range(NT // 2):
                    hN_t = hN_tiles[j]
                    for jj in range(2):
                        i = 2 * j + jj
                        wbig = scr.tile([128, 128], BF16, tag="wbig")
                        nc.vector.tensor_scalar_mul(
                            wbig[:], dwide_sb[:, 120 - 8 * i:248 - 8 * i],
                            wN[:, i:i + 1])
                        for hc in range(4):
                            nc.tensor.matmul(
                                ps_e[:, hc * 512:(hc + 1) * 512],
                                wbig[:], hN_t[:, jj, hc * 512:(hc + 1) * 512],
                                start=(i == 0), stop=(i == NT - 1))
                for hc in range(4):
                    nc.vector.tensor_copy(entries[:, hc * 512:(hc + 1) * 512],
                                          ps_e[:, hc * 512:(hc + 1) * 512])
                for kg in range(KT // 4):
                    ps_t = ptp2.tile([128, 4, 128], BF16, tag="tp_e")
                    for kk in range(4):
                        k = kg * 4 + kk
                        nc.tensor.transpose(ps_t[:, kk, :],
                                            entries[:, k * 128:(k + 1) * 128],
                                            ident_bf[:])
                    nc.vector.tensor_copy(eT[:, kg * 4:(kg + 1) * 4, :],
                                          ps_t[:])

                ps_kv = pkv.tile([C, 256], F32)
                for k in range(KT):
                    nc.tensor.matmul(ps_kv[:], eT[:, k, :], wkv_sb[:, k, :],
                                     start=(k == 0),
                                     stop=(zero_bias and k == KT - 1))
                if not zero_bias:
                    nc.tensor.matmul(ps_kv[:], ones1[:], b_kv_sb[:],
                                     start=False, stop=True)

                # ck: rmsnorm + rope at block-end positions
                ssqc = sts.tile([C, 1], F32)
                sq_c = scr.tile([C, 128], F32, tag="sq")
                nc.scalar.activation(sq_c[:], ps_kv[:, 0:128],
                                     mybir.ActivationFunctionType.Square,
                                     accum_out=ssqc[:])
                rmsc = sts.tile([C, 1], F32)
                nc.scalar.activation(rmsc[:], ssqc[:],
                                     mybir.ActivationFunctionType.Sqrt,
                                     scale=1.0 / HD, bias=eps_t[:])
                rinvc = sts.tile([C, 1], F32)
                nc.vector.reciprocal(rinvc[:], rmsc[:])
                ckn = scr.tile([C, 128], F32, tag="ckn")
                nc.vector.tensor_scalar_mul(ckn[:], ps_kv[:, 0:128], rinvc[:])
                ckR = scr.tile([C, 128], BF16, tag="ckR")
                ct1 = scr.tile([C, HALF], F32, tag="ct1")
                ct2 = scr.tile([C, HALF], F32, tag="ct2")
                nc.vector.tensor_mul(ct1[:], ckn[:, 0:HALF], ctA_sb)
                nc.vector.tensor_mul(ct2[:], ckn[:, HALF:ROPE], ctB_sb)
                nc.vector.tensor_sub(ckR[:, 0:HALF], ct1[:], ct2[:])
                nc.vector.tensor_mul(ct1[:], ckn[:, 0:HALF], ctC_sb)
                nc.vector.tensor_mul(ct2[:], ckn[:, HALF:ROPE], ctD_sb)
                nc.vector.tensor_add(ckR[:, HALF:ROPE], ct1[:], ct2[:])
                nc.vector.tensor_mul(ckR[:, ROPE:128], ckn[:, ROPE:128],
                                     ck_pass_sb)
                nc.vector.tensor_copy(cvn[:], ps_kv[:, 128:256])
                ps_ct = ptp2.tile([128, 128], BF16, tag="tp_e")
                nc.tensor.transpose(ps_ct[:], ckR[:], ident_bf[:])
                for h in range(HPC):
                    nc.vector.tensor_copy(ckT_aug[:, h, 0:128], ps_ct[:])

            # ============ P3+P4: attention + out projection ============
            with (
                tc.tile_pool(name="ps_sc", bufs=2, space="PSUM") as psc,
                tc.tile_pool(name="ps_ctx", bufs=2, space="PSUM") as pcx,
                tc.tile_pool(name="ps_tp3", bufs=2, space="PSUM") as ptp3,
                tc.tile_pool(name="ps_out", bufs=1, space="PSUM") as pout,
            ):
                def front(i, h):
                    qT = qkT[:, i + 1, h * 128:(h + 1) * 128]
                    ps_s = psc.tile([128, 385], F32, tag="ps_s")
                    # mask pre-loaded into psum via PE (ident.T @ mcomb_i),
                    # scores accumulate on top
                    nc.tensor.matmul(ps_s[:], ident_bf[:],
                                     mcomb_sb[:, i, :],
                                     start=True, stop=False)
                    nc.tensor.matmul(
                        ps_s[:, 0:256], qT,
                        qkT[:, i:i + 2, 256 + h * 128:256 + (h + 1) * 128],
                        start=False, stop=True)
                    nc.tensor.matmul(ps_s[:, 256:385], qT,
                                     ckT_aug[:, h, :],
                                     start=False, stop=True)
                    # two separate softmaxes: local (cols 0:256) and
                    # compressed+sink (cols 256:385)
                    p_t = scr.tile([128, 385], BF16, tag="p", bufs=4)
                    den = sts.tile([128, 2], F32)
                    nc.scalar.activation(p_t[:, 0:256], ps_s[:, 0:256],
                                         mybir.ActivationFunctionType.Exp,
                                         scale=SCALE,
                                         accum_out=den[:, 0:1])
                    nc.scalar.activation(p_t[:, 256:385], ps_s[:, 256:385],
                                         mybir.ActivationFunctionType.Exp,
                                         scale=SCALE,
                                         accum_out=den[:, 1:2])
                    return p_t, den

                def back(i, h, p_t, den):
                    rden = sts.tile([128, 2], F32)
                    nc.vector.reciprocal(rden[:], den[:])
                    # normalization fused into the p-transposes:
                    # matmul(out, x, diag(r)) = x.T @ diag(r)
                    rdg = scr.tile([128, 256], BF16, tag="rdg", bufs=3)
                    nc.vector.tensor_scalar_mul(rdg[:, 0:128],
                                                ident_bf[:],
                                                rden[:, 0:1])
                    nc.vector.tensor_scalar_mul(rdg[:, 128:256],
                                                ident_bf[:],
                                                rden[:, 1:2])
                    ps_tp_t = ptp3.tile([128, 512], F32, tag="tp3")
                    for cidx in range(3):
                        nc.tensor.matmul(
                            ps_tp_t[:, cidx * 128:(cidx + 1) * 128],
                            p_t[:, cidx * 128:(cidx + 1) * 128],
                            rdg[:, 0:128] if cidx < 2 else rdg[:, 128:256],
                            start=True, stop=True)
                    nc.tensor.matmul(ps_tp_t[0:1, 384:512],
                                     p_t[:, 384:385], rdg[:, 128:256],
                                     start=True, stop=True)
                    pT_sb = scr.tile([128, 512], BF16, tag="pT", bufs=4)
                    if h == 0:
                        nc.vector.tensor_copy(pT_sb[:], ps_tp_t[:])
                    else:
                        nc.scalar.copy(pT_sb[:], ps_tp_t[:])
                    pT_sink = pT_sb[0:1, 384:512]

                    ps_c = pcx.tile([128, 128], F32, tag="ps_c")
                    nc.tensor.matmul(ps_c[:],
                                     lvn[:, i - 1 if i else NT - 1,
                                         h * 128:(h + 1) * 128],
                                     pT_sb[:, 0:128],
                                     start=True, stop=False)
                    nc.tensor.matmul(ps_c[:],
                                     lvn[:, i, h * 128:(h + 1) * 128],
                                     pT_sb[:, 128:256],
                                     start=False, stop=False)
                    nc.tensor.matmul(ps_c[:], cvn[:], pT_sb[:, 256:384],
                                     start=False, stop=False)
                    nc.tensor.matmul(
                        ps_c[:],
                        sinkv_sb[:, h * 128:(h + 1) * 128],
                        pT_sink, start=False, stop=True)
                    nc.vector.tensor_copy(
                        mgT[:, h, i * 128:(i + 1) * 128], ps_c[:])

                def outproj(i):
                    # out projection for s-tile i (0.5 folded into wo)
                    o_sb = scr.tile([128, HID], BF16, tag="o_sb", bufs=2)
                    for half in range(2):
                        ps_o = pout.tile([128, HID // 2], F32, tag="ps_o")
                        for h in range(HPC):
                            for oc in range(2):
                                occ = half * 2 + oc
                                nc.tensor.matmul(
                                    ps_o[:, oc * 512:(oc + 1) * 512],
                                    mgT[:, h, i * 128:(i + 1) * 128],
                                    wo_sb[:, h, occ * 512:(occ + 1) * 512],
                                    start=(h == 0), stop=(h == HPC - 1))
                        dst = o_sb[:, half * 1024:(half + 1) * 1024]
                        if half == 0:
                            nc.vector.tensor_copy(dst, ps_o[:])
                        else:
                            nc.scalar.copy(dst, ps_o[:])
                    nc.sync.dma_start(out=out_p.ap()[i], in_=o_sb[:])

                # software pipeline: front(n) ... back(n-1) ... outproj(i)
                pend = None
                for i in range(NT):
                    for h in range(HPC):
                        cur = (i, h, *front(i, h))
                        if pend is not None:
                            back(*pend)
                            if pend[1] == HPC - 1:
                                outproj(pend[0])
                        pend = cur
                back(*pend)
                outproj(pend[0])

    nc.compile()
    return nc
